# revision 4
# baseline (speedup 1.0000x reference)
"""Chamfer distance L2 kernel for Trainium2, 8 NeuronCores.

Problem: xyz1, xyz2 [B=4, N=8192, 3] fp32. Output: scalar
mean_i(min_j ||x1_i - x2_j||^2) + mean_j(min_i ||x1_i - x2_j||^2).

Decomposition: 8 independent jobs = (batch, direction), one per NeuronCore.
Each job: for 8192 query points, exact min squared distance to 8192
candidates.

Algorithm (exact, 2-round candidate pruning):
  * Host orders each job's queries with a k-d median partition (leaves of
    16) so each block of 128 consecutive queries is 8 compact sub-boxes.
  * For each block, host gathers the W=512 candidates nearest to the block
    (by min distance to the 8 sub-bboxes, a lower bound on any
    query-candidate distance) and records r2_cov = the (W+1)-th smallest
    such bound.
  * Device (round 1) computes per-query min over the gathered candidates:
    one K=15 matmul per block emits pairwise squared distances into PSUM
    (bf16 hi/lo compensated products accumulated in fp32, ~2e-4 abs err),
    VectorE reduce_min produces the row mins.
  * Host verifies per query: if ub^2 + pad <= r2_cov^2, every non-gathered
    candidate is provably farther than the best found -> exact. Queries
    failing the test ("stragglers") are regrouped into new blocks; all
    candidates within their ub-balls (again bounded via sub-bboxes) are
    chunked into W-sized slots and run through the same compiled NEFF in
    one or more extra waves; host min-combines. Round 2 is conclusive:
    every candidate that could beat the round-1 upper bound is included.

The device does all distance arithmetic; the host only sorts/gathers by
coordinate bounds (O(N) per block) and combines results.

Pairwise row content (query-side |a|^2 is added on host after the min; the
max(.,0) clip also commutes with the min):
   k 0..2 : (-2*a_hi) * b_hi      k 3..5 : (-2*a_hi) * b_lo
   k 6..8 : (-2*a_lo) * b_hi      k 9..11: (-2*a_lo) * b_lo
   k12..14: 1 * sqB_{hi,lo,lo2}
"""

import numpy as np
import ml_dtypes

import concourse.bass as bass
import concourse.tile as tile
from concourse import bacc, mybir
from concourse.bass_utils import run_bass_kernel_spmd

BF16 = ml_dtypes.bfloat16
F32 = np.float32

K = 15            # augmented contraction rows
W = 512           # candidates per slot
NSLOT = 64        # slots per core per invocation
GRP = 4           # slots fused per DMA + reduce (4 PSUM banks)
LEAF = 16         # k-d leaf size -> 8 sub-bboxes per 128-query block
ERR_PAD = 1e-3    # abs error bound on device pairwise d^2 (worst case ~6e-4)
N_CORES = 8


# --------------------------------------------------------------------------
# Device program (one static NEFF, SPMD on 8 cores)
# --------------------------------------------------------------------------

def build_kernel():
    nc = bacc.Bacc("TRN2", target_bir_lowering=False, debug=False)

    lhsT_d = nc.dram_tensor("lhsT", [K, NSLOT * 128], mybir.dt.bfloat16,
                            kind="ExternalInput")
    rhs_d = nc.dram_tensor("rhs", [NSLOT // GRP, K, GRP * W], mybir.dt.bfloat16,
                           kind="ExternalInput")
    out_d = nc.dram_tensor("mins", [128, NSLOT], mybir.dt.float32,
                           kind="ExternalOutput")

    with tile.TileContext(nc) as tc:
        with (
            tc.tile_pool(name="io", bufs=1) as io_pool,
            tc.tile_pool(name="rh", bufs=4) as rh_pool,
            tc.tile_pool(name="ps", bufs=2, space=bass.MemorySpace.PSUM) as ps_pool,
        ):
            lhsT_s = io_pool.tile([K, NSLOT * 128], mybir.dt.bfloat16)
            nc.sync.dma_start(lhsT_s[:], lhsT_d[:])
            mins_all = io_pool.tile([128, NSLOT], mybir.dt.float32)

            for g in range(NSLOT // GRP):
                rt = rh_pool.tile([K, GRP * W], mybir.dt.bfloat16)
                nc.sync.dma_start(rt[:], rhs_d[g])
                ps = ps_pool.tile([128, GRP * W], mybir.dt.float32)
                for s in range(GRP):
                    m = g * GRP + s
                    nc.tensor.matmul(
                        ps[:, s * W : (s + 1) * W],
                        lhsT_s[:, m * 128 : (m + 1) * 128],
                        rt[:, s * W : (s + 1) * W],
                    )
                nc.vector.tensor_reduce(
                    mins_all[:, g * GRP : (g + 1) * GRP],
                    ps[:].rearrange("p (s n) -> p s n", n=W),
                    axis=mybir.AxisListType.X,
                    op=mybir.AluOpType.min,
                )

            nc.sync.dma_start(out_d[:], mins_all[:])

    nc.compile()
    return nc


_NC_CACHE = {}


def _get_nc():
    if "nc" not in _NC_CACHE:
        _NC_CACHE["nc"] = build_kernel()
    return _NC_CACHE["nc"]


def run_wave(in_maps, trace=False, **kw):
    nc = _get_nc()
    return run_bass_kernel_spmd(nc, in_maps, list(range(N_CORES)), trace=trace, **kw)


# --------------------------------------------------------------------------
# Host-side prep
# --------------------------------------------------------------------------

def _split2(x):
    h = x.astype(BF16)
    l = (x - h.astype(F32)).astype(BF16)
    return h, l


def kd_order(P, leaf=LEAF):
    """Permutation grouping points into contiguous compact leaves of `leaf`."""
    out = []

    def rec(ids):
        if len(ids) <= leaf:
            out.append(ids)
            return
        pts = P[ids]
        ax = int(np.argmax(pts.max(0) - pts.min(0)))
        k = len(ids) // 2
        part = np.argpartition(pts[:, ax], k)
        rec(ids[part[:k]])
        rec(ids[part[k:]])

    rec(np.arange(len(P)))
    return np.concatenate(out)


def subbox_d2(q, Bd, nsub):
    """Min squared distance from every candidate to nsub sub-bboxes of q.

    Lower bounds dist(query, candidate)^2 for every query in q.
    """
    d2 = np.full(len(Bd), np.inf)
    sub = max(1, len(q) // nsub)
    for j0 in range(0, len(q), sub):
        qq = q[j0 : j0 + sub]
        lo, hi = qq.min(0), qq.max(0)
        d_ax = np.maximum(np.maximum(lo[None, :] - Bd, Bd - hi[None, :]), 0.0)
        d2 = np.minimum(d2, (d_ax ** 2).sum(-1))
    return d2


class Job:
    """Host state for one (queries, candidates) job."""

    def __init__(self, Aq, Bc):
        self.N = len(Aq)
        self.order = kd_order(Aq)
        A = Aq[self.order]
        self.Ad = A.astype(np.float64)
        self.Bd = Bc.astype(np.float64)

        ah, al = _split2(A)
        m2ah = (ah.astype(F32) * -2.0).astype(BF16)
        m2al = (al.astype(F32) * -2.0).astype(BF16)
        L = np.empty((K, self.N), BF16)
        L[0:3] = m2ah.T
        L[3:6] = m2ah.T
        L[6:9] = m2al.T
        L[9:12] = m2al.T
        L[12:15] = np.ones((3, self.N), BF16)
        self.Lrows = L

        bh, bl = _split2(Bc)
        sqB = (self.Bd ** 2).sum(-1).astype(F32)
        s0 = sqB.astype(BF16)
        r = sqB - s0.astype(F32)
        s1 = r.astype(BF16)
        s2 = (r - s1.astype(F32)).astype(BF16)
        R = np.empty((K, self.N), BF16)
        R[0:3] = bh.T
        R[3:6] = bl.T
        R[6:9] = bh.T
        R[9:12] = bl.T
        R[12] = s0
        R[13] = s1
        R[14] = s2
        self.Rrows = R

        self.sqA = (self.Ad ** 2).sum(-1)  # permuted order, float64
        self.mins = np.full(self.N, np.inf)  # device d2 minus sqA term

        # Round-1 block gather
        self.sel = np.empty((self.N // 128, W), np.int64)
        self.r2cov = np.empty(self.N // 128)
        for m in range(self.N // 128):
            q = self.Ad[m * 128 : (m + 1) * 128]
            d2b = subbox_d2(q, self.Bd, 128 // LEAF)
            part = np.argpartition(d2b, W)
            self.sel[m] = part[:W]
            self.r2cov[m] = d2b[part[W]]

    def round1_slots(self):
        return [
            (self.order_slice(m), self.sel[m])
            for m in range(self.N // 128)
        ]

    def order_slice(self, m):
        return np.arange(m * 128, (m + 1) * 128)

    def absorb(self, qidx, vals):
        np.minimum.at(self.mins, qidx, vals.astype(np.float64))

    def stragglers(self):
        """Per-query coverage check after round 1."""
        ub2 = np.maximum(self.mins + self.sqA, 0.0) + ERR_PAD
        r2 = np.repeat(self.r2cov, 128)
        return np.where(ub2 > r2)[0]

    def round2_slots(self, strag):
        """Conclusive follow-up slots for straggler queries."""
        slots = []
        if len(strag) == 0:
            return slots
        sord = strag[kd_order(self.Ad[strag])]
        for m0 in range(0, len(sord), 128):
            ids = sord[m0 : m0 + 128]
            q = self.Ad[ids]
            nsub = max(1, len(ids) // LEAF)
            d2b = subbox_d2(q, self.Bd, nsub)
            ub2max = (np.maximum(self.mins[ids] + self.sqA[ids], 0.0) + ERR_PAD).max()
            need = np.where(d2b <= ub2max)[0]
            if len(need) == 0:
                continue
            for c0 in range(0, len(need), W):
                cand = need[c0 : c0 + W]
                if len(cand) < W:
                    cand = np.concatenate(
                        [cand, np.full(W - len(cand), cand[0], np.int64)])
                slots.append((ids, cand))
        return slots


def _assemble_core(slots):
    """Build one core's in_map from up to NSLOT (job, qidx, cand) slots."""
    lhsT = np.zeros((K, NSLOT * 128), BF16)
    rhs = np.zeros((NSLOT // GRP, K, GRP * W), BF16)
    meta = []
    for s, (job, qidx, cand) in enumerate(slots):
        ncol = len(qidx)
        lhsT[:, s * 128 : s * 128 + ncol] = job.Lrows[:, qidx]
        g, r = divmod(s, GRP)
        rhs[g, :, r * W : (r + 1) * W] = job.Rrows[:, cand]
        meta.append((job, qidx))
    return {"lhsT": lhsT, "rhs": rhs}, meta


def _run_waves(all_slots, trace=False):
    """Pack slots onto cores, run as many 8-core waves as needed."""
    results = []
    per_wave = N_CORES * NSLOT
    for w0 in range(0, len(all_slots), per_wave):
        wave = all_slots[w0 : w0 + per_wave]
        in_maps = []
        metas = []
        for c in range(N_CORES):
            cslots = wave[c * NSLOT : (c + 1) * NSLOT]
            if cslots:
                im, meta = _assemble_core(cslots)
            else:
                im, meta = _assemble_core([])
            in_maps.append(im)
            metas.append(meta)
        res = run_wave(in_maps, trace=trace)
        results.append(res)
        for c in range(N_CORES):
            mins = res.results[c]["mins"]  # [128, NSLOT]
            for s, (job, qidx) in enumerate(metas[c]):
                job.absorb(qidx, mins[: len(qidx), s])
    return results


def kernel(xyz1, xyz2):
    xyz1 = np.asarray(xyz1, F32)
    xyz2 = np.asarray(xyz2, F32)
    nb = xyz1.shape[0]

    jobs = []
    for b in range(nb):
        jobs.append(Job(xyz1[b], xyz2[b]))
        jobs.append(Job(xyz2[b], xyz1[b]))

    # Round 1: job j on core j
    slots1 = []
    for j in jobs:
        for qidx, cand in j.round1_slots():
            slots1.append((j, qidx, cand))
    # order is job-major: core c gets job c's 64 slots
    _run_waves(slots1)

    # Round 2: conclusive straggler slots (usually one short wave)
    slots2 = []
    for j in jobs:
        for qidx, cand in j.round2_slots(j.stragglers()):
            slots2.append((j, qidx, cand))
    if slots2:
        _run_waves(slots2)

    total = 0.0
    for j in jobs:
        d = np.maximum(j.mins + j.sqA, 0.0)
        total += d.mean() / nb
    return np.asarray(total, dtype=F32)


# revision 5
# speedup vs baseline: 1.3543x; 1.3543x over previous
"""Chamfer distance L2 kernel for Trainium2, 8 NeuronCores.

Problem: xyz1, xyz2 [B=4, N=8192, 3] fp32. Output: scalar
mean_i(min_j ||x1_i - x2_j||^2) + mean_j(min_i ||x1_i - x2_j||^2).

Decomposition: 8 independent jobs = (batch, direction), one per NeuronCore.
Each job: for 8192 query points, exact min squared distance to 8192
candidates.

Algorithm (exact, 2-round candidate pruning):
  * Host orders each job's queries with a k-d median partition (leaves of
    8) so each block of 128 consecutive queries is 16 compact sub-boxes.
  * For each block, host gathers the W=512 candidates nearest to the block
    (by min squared distance to the sub-bboxes -- a lower bound on any
    query-candidate distance) and records, per leaf, the smallest bound
    among NON-gathered candidates (the leaf's coverage radius rcov).
  * Device (round 1) computes per-query min over the gathered candidates:
    one K=15 matmul per block emits pairwise squared distances into PSUM
    (bf16 hi/lo compensated products accumulated in fp32; the query-side
    |a|^2 term is constant per row and added on the host after the min,
    which also lets max(.,0) commute out), then VectorE reduce_min
    produces the row mins (4 blocks per fused reduce across 4 PSUM banks).
  * Host verifies per query: if device_min + |a|^2 + pad(q) <= rcov(leaf),
    every non-gathered candidate is provably farther than the best found
    -> exact. pad(q) soundly bounds the device arithmetic error
    (~2.5e-5*|a|^2 + 2e-5). Queries failing the test ("stragglers") are
    regrouped; all candidates within their upper-bound balls (bounded via
    sub-bboxes again) are chunked into W-sized slots and run through a
    second, smaller compiled NEFF; host min-combines. Round 2 is
    conclusive -- every candidate that could beat the round-1 bound is
    included -- so no further verification is needed.

The device does all distance arithmetic; the host only sorts/gathers by
coordinate bounds and combines results.

Pairwise matmul row content (K=15):
   k 0..2 : (-2*a_hi) * b_hi      k 3..5 : (-2*a_hi) * b_lo
   k 6..8 : (-2*a_lo) * b_hi      k 9..11: (-2*a_lo) * b_lo
   k12..14: 1 * sqB_{hi,lo,lo2}
bf16*bf16 products are exact in fp32, so the dominant error is the dropped
sub-bf16 residue of the splits, ~1e-4 absolute on d^2.
"""

import numpy as np
import ml_dtypes

import concourse.bass as bass
import concourse.tile as tile
from concourse import bacc, mybir
from concourse.bass_utils import run_bass_kernel_spmd

BF16 = ml_dtypes.bfloat16
F32 = np.float32

K = 15            # augmented contraction rows
W = 512           # candidates per slot
NSLOT1 = 64       # slots per core, round-1 NEFF
NSLOT2 = 24       # slots per core, straggler NEFF
GRP = 4           # slots fused per DMA + reduce (4 PSUM banks)
LEAF = 8          # k-d leaf size -> 16 sub-bboxes per 128-query block
N_CORES = 8

# Sound per-query bound on device pairwise-d^2 arithmetic error:
# split residues ~2^-16*|a||b| + fp32 PSUM accumulation ~K*2^-23*|partials|.
PAD_SCALE = 2.5e-5
PAD_ABS = 2e-5


def _pad_q(sqA):
    return PAD_SCALE * sqA + PAD_ABS


# --------------------------------------------------------------------------
# Device program (static NEFFs, SPMD on 8 cores)
# --------------------------------------------------------------------------

def build_kernel(nslot):
    nc = bacc.Bacc("TRN2", target_bir_lowering=False, debug=False)

    lhsT_d = nc.dram_tensor("lhsT", [K, nslot * 128], mybir.dt.bfloat16,
                            kind="ExternalInput")
    rhs_d = nc.dram_tensor("rhs", [nslot // GRP, K, GRP * W], mybir.dt.bfloat16,
                           kind="ExternalInput")
    out_d = nc.dram_tensor("mins", [128, nslot], mybir.dt.float32,
                           kind="ExternalOutput")

    with tile.TileContext(nc) as tc:
        with (
            tc.tile_pool(name="io", bufs=1) as io_pool,
            tc.tile_pool(name="rh", bufs=4) as rh_pool,
            tc.tile_pool(name="ps", bufs=2, space=bass.MemorySpace.PSUM) as ps_pool,
        ):
            lhsT_s = io_pool.tile([K, nslot * 128], mybir.dt.bfloat16)
            nc.sync.dma_start(lhsT_s[:], lhsT_d[:])
            mins_all = io_pool.tile([128, nslot], mybir.dt.float32)

            for g in range(nslot // GRP):
                rt = rh_pool.tile([K, GRP * W], mybir.dt.bfloat16)
                nc.sync.dma_start(rt[:], rhs_d[g])
                ps = ps_pool.tile([128, GRP * W], mybir.dt.float32)
                for s in range(GRP):
                    m = g * GRP + s
                    nc.tensor.matmul(
                        ps[:, s * W : (s + 1) * W],
                        lhsT_s[:, m * 128 : (m + 1) * 128],
                        rt[:, s * W : (s + 1) * W],
                    )
                nc.vector.tensor_reduce(
                    mins_all[:, g * GRP : (g + 1) * GRP],
                    ps[:].rearrange("p (s n) -> p s n", n=W),
                    axis=mybir.AxisListType.X,
                    op=mybir.AluOpType.min,
                )

            nc.sync.dma_start(out_d[:], mins_all[:])

    nc.compile()
    return nc


_NC_CACHE = {}


def _get_nc(nslot):
    if nslot not in _NC_CACHE:
        _NC_CACHE[nslot] = build_kernel(nslot)
    return _NC_CACHE[nslot]


def run_wave(in_maps, nslot=NSLOT1, trace=False, **kw):
    nc = _get_nc(nslot)
    return run_bass_kernel_spmd(nc, in_maps, list(range(N_CORES)), trace=trace, **kw)


# --------------------------------------------------------------------------
# Host-side prep
# --------------------------------------------------------------------------

def _split2(x):
    h = x.astype(BF16)
    l = (x - h.astype(F32)).astype(BF16)
    return h, l


def kd_order(P, leaf=LEAF):
    """Permutation grouping points into contiguous compact leaves of `leaf`."""
    out = []

    def rec(ids):
        if len(ids) <= leaf:
            out.append(ids)
            return
        pts = P[ids]
        ax = int(np.argmax(pts.max(0) - pts.min(0)))
        k = len(ids) // 2
        part = np.argpartition(pts[:, ax], k)
        rec(ids[part[:k]])
        rec(ids[part[k:]])

    rec(np.arange(len(P)))
    return np.concatenate(out)


def leaf_d2(q, Bd, leaf=LEAF):
    """[nleaf, ncand] min squared distance from candidates to each leaf bbox.

    Lower bounds dist(query, candidate)^2 for every query of that leaf.
    """
    nleaf = (len(q) + leaf - 1) // leaf
    lo = np.empty((nleaf, 3))
    hi = np.empty((nleaf, 3))
    for j in range(nleaf):
        qq = q[j * leaf : (j + 1) * leaf]
        lo[j] = qq.min(0)
        hi[j] = qq.max(0)
    d_ax = np.maximum(
        np.maximum(lo[:, None, :] - Bd[None, :, :], Bd[None, :, :] - hi[:, None, :]),
        0.0,
    )
    return (d_ax ** 2).sum(-1)


class Job:
    """Host state for one (queries, candidates) job."""

    def __init__(self, Aq, Bc):
        self.N = len(Aq)
        self.order = kd_order(Aq)
        A = Aq[self.order]
        self.Ad = A.astype(np.float64)
        self.Bd = Bc.astype(np.float64)

        ah, al = _split2(A)
        m2ah = (ah.astype(F32) * -2.0).astype(BF16)
        m2al = (al.astype(F32) * -2.0).astype(BF16)
        L = np.empty((K, self.N), BF16)
        L[0:3] = m2ah.T
        L[3:6] = m2ah.T
        L[6:9] = m2al.T
        L[9:12] = m2al.T
        L[12:15] = np.ones((3, self.N), BF16)
        self.Lrows = L

        bh, bl = _split2(Bc)
        sqB = (self.Bd ** 2).sum(-1).astype(F32)
        s0 = sqB.astype(BF16)
        r = sqB - s0.astype(F32)
        s1 = r.astype(BF16)
        s2 = (r - s1.astype(F32)).astype(BF16)
        R = np.empty((K, self.N), BF16)
        R[0:3] = bh.T
        R[3:6] = bl.T
        R[6:9] = bh.T
        R[9:12] = bl.T
        R[12] = s0
        R[13] = s1
        R[14] = s2
        self.Rrows = R

        self.sqA = (self.Ad ** 2).sum(-1)  # permuted order, float64
        self.mins = np.full(self.N, np.inf)  # device value: d2 - sqA

        # Round-1 gather: per block, W nearest-by-leaf-bbox candidates;
        # per leaf, coverage radius = min bound among non-gathered.
        nblk = self.N // 128
        nsub = 128 // LEAF
        self.sel = np.empty((nblk, W), np.int64)
        self.rcov = np.empty(nblk * nsub)
        for m in range(nblk):
            q = self.Ad[m * 128 : (m + 1) * 128]
            d2bs = leaf_d2(q, self.Bd)          # [nsub, ncand]
            d2b = d2bs.min(0)
            part = np.argpartition(d2b, W)
            self.sel[m] = part[:W]
            unsel = np.ones(self.N, bool)
            unsel[part[:W]] = False
            self.rcov[m * nsub : (m + 1) * nsub] = d2bs[:, unsel].min(1)

    def round1_slots(self):
        return [
            (np.arange(m * 128, (m + 1) * 128), self.sel[m])
            for m in range(self.N // 128)
        ]

    def absorb(self, qidx, vals):
        np.minimum.at(self.mins, qidx, vals.astype(np.float64))

    def stragglers(self):
        """Per-query coverage check after round 1."""
        ub2 = np.maximum(self.mins + self.sqA, 0.0) + _pad_q(self.sqA)
        return np.where(ub2 > np.repeat(self.rcov, LEAF))[0]

    def round2_slots(self, strag):
        """Conclusive follow-up slots for straggler queries."""
        slots = []
        if len(strag) == 0:
            return slots
        sord = strag[kd_order(self.Ad[strag])]
        for m0 in range(0, len(sord), 128):
            ids = sord[m0 : m0 + 128]
            q = self.Ad[ids]
            d2b = leaf_d2(q, self.Bd).min(0)
            ub2max = (np.maximum(self.mins[ids] + self.sqA[ids], 0.0)
                      + _pad_q(self.sqA[ids])).max()
            need = np.where(d2b <= ub2max)[0]
            if len(need) == 0:
                continue
            for c0 in range(0, len(need), W):
                cand = need[c0 : c0 + W]
                if len(cand) < W:
                    cand = np.concatenate(
                        [cand, np.full(W - len(cand), cand[0], np.int64)])
                slots.append((ids, cand))
        return slots


def _assemble_core(slots, nslot):
    """Build one core's in_map from up to `nslot` (job, qidx, cand) slots."""
    lhsT = np.zeros((K, nslot * 128), BF16)
    rhs = np.zeros((nslot // GRP, K, GRP * W), BF16)
    meta = []
    for s, (job, qidx, cand) in enumerate(slots):
        ncol = len(qidx)
        lhsT[:, s * 128 : s * 128 + ncol] = job.Lrows[:, qidx]
        g, r = divmod(s, GRP)
        rhs[g, :, r * W : (r + 1) * W] = job.Rrows[:, cand]
        meta.append((job, qidx))
    return {"lhsT": lhsT, "rhs": rhs}, meta


def _run_waves(all_slots, nslot, trace=False):
    """Pack slots onto cores, run as many 8-core waves as needed."""
    per_wave = N_CORES * nslot
    for w0 in range(0, len(all_slots), per_wave):
        wave = all_slots[w0 : w0 + per_wave]
        in_maps = []
        metas = []
        for c in range(N_CORES):
            cslots = wave[c * nslot : (c + 1) * nslot]
            im, meta = _assemble_core(cslots, nslot)
            in_maps.append(im)
            metas.append(meta)
        res = run_wave(in_maps, nslot=nslot, trace=trace)
        for c in range(N_CORES):
            mins = res.results[c]["mins"]  # [128, nslot]
            for s, (job, qidx) in enumerate(metas[c]):
                job.absorb(qidx, mins[: len(qidx), s])


def kernel(xyz1, xyz2):
    xyz1 = np.asarray(xyz1, F32)
    xyz2 = np.asarray(xyz2, F32)
    nb = xyz1.shape[0]

    jobs = []
    for b in range(nb):
        jobs.append(Job(xyz1[b], xyz2[b]))
        jobs.append(Job(xyz2[b], xyz1[b]))

    # Round 1: job j's 64 blocks on core j (slot list is job-major)
    slots1 = [(j, q, c) for j in jobs for q, c in j.round1_slots()]
    _run_waves(slots1, NSLOT1)

    # Round 2: conclusive straggler slots (typically one short wave)
    slots2 = [(j, q, c) for j in jobs for q, c in j.round2_slots(j.stragglers())]
    if slots2:
        nslot = NSLOT2 if len(slots2) <= N_CORES * NSLOT2 else NSLOT1
        _run_waves(slots2, nslot)

    total = 0.0
    for j in jobs:
        d = np.maximum(j.mins + j.sqA, 0.0)
        total += d.mean() / nb
    return np.asarray(total, dtype=F32)
